# revision 6
# baseline (speedup 1.0000x reference)
"""GQA (B=1, S=2048, D=2048, 32 Q heads / 8 KV heads, head_dim=64, RoPE,
non-causal softmax) on 8 Trainium2 NeuronCores.

Sharding: tensor-parallel over heads. Core c owns Q heads 4c..4c+3 and KV head c.
Each core computes y_c = softmax(q_c k_c^T / 8) v_c @ Wo[:, c*256:(c+1)*256].T
(a full [S, D] partial); the host sums the 8 partials.

On-chip layout is fully transposed ("T" = [feature, seq]):
  qT = WqT.T @ xT          (PE, f32r)      [256, S]  (4 heads)
  kvT = WkvT.T @ xT        (PE, f32r)      [128, S]  (k rows 0:64, v rows 64:128)
  RoPE on qT/kT            (DVE, partition-shifted reads)
  v1[kt] = T(vT chunk)|1   (PE transpose + ACT copies)  [128, 65]
  sT = kTr.T_chunk @ qTr   (PE)            scores transposed [s_k, s_q]
  PT = exp(sT/8)           (ACT, f32r out)
  pv = v1.T @ PT           (PE, accumulate over s_k; row 64 = softmax denom l)
  outT = pv[0:64] * bcast(1/l)  (DVE; bcast via K=1 matmul)
  y = outT_packed.T @ WoT  (PE)
All matmuls run in float32r (TF32-like, ~2e-4 rel err) at 1 cycle/row.
"""

import numpy as np

S = 2048
D = 2048
HD = 64
N_CORES = 8
Q_PER_CORE = 4  # 256 o-dims per core
ROPE_BASE = 10000.0

_cached = {}


def _build_program():
    import concourse.bass as bass
    import concourse.mybir as mybir
    import concourse.tile as tile
    from concourse import bacc

    F32R, F32 = mybir.dt.float32r, mybir.dt.float32
    EXP = mybir.ActivationFunctionType.Exp

    nc = bacc.Bacc("TRN2", target_bir_lowering=False, debug=False)

    xT = nc.dram_tensor("xT", [D, S], F32R, kind="ExternalInput").ap()
    wqt = nc.dram_tensor("wqt", [D, 256], F32R, kind="ExternalInput").ap()
    wkvt = nc.dram_tensor("wkvt", [D, 128], F32R, kind="ExternalInput").ap()
    wot = nc.dram_tensor("wot", [256, D], F32R, kind="ExternalInput").ap()
    cos2 = nc.dram_tensor("cos2", [128, S], F32, kind="ExternalInput").ap()
    sin2s = nc.dram_tensor("sin2s", [128, S], F32, kind="ExternalInput").ap()
    ones1 = nc.dram_tensor("ones1", [1, 64], F32R, kind="ExternalInput").ap()
    onescol = nc.dram_tensor("onescol", [128, 1], F32R, kind="ExternalInput").ap()
    ident = nc.dram_tensor("ident", [64, 64], F32R, kind="ExternalInput").ap()
    y = nc.dram_tensor("y", [S, D], F32, kind="ExternalOutput").ap()

    with tile.TileContext(nc) as tc:
        with tc.tile_pool(name="singles", bufs=1) as singles, \
             tc.tile_pool(name="xtp", bufs=2) as xtp, \
             tc.tile_pool(name="pcp", bufs=3) as pcp, \
             tc.tile_pool(name="rope", bufs=3) as rope, \
             tc.tile_pool(name="persist", bufs=1) as persist, \
             tc.tile_pool(name="vtcp", bufs=2) as vtcp, \
             tc.tile_pool(name="ptp", bufs=3) as ptp, \
             tc.tile_pool(name="rcp", bufs=2) as rcp, \
             tc.tile_pool(name="othp", bufs=3) as othp, \
             tc.tile_pool(name="ysbp", bufs=2) as ysbp, \
             tc.tile_pool(name="pp", bufs=2, space="PSUM") as pp, \
             tc.tile_pool(name="pss", bufs=3, space="PSUM") as pss, \
             tc.tile_pool(name="pspv", bufs=2, space="PSUM") as pspv, \
             tc.tile_pool(name="psb", bufs=1, space="PSUM") as psb, \
             nc.allow_low_precision(reason="f32r rounding is intended"):

            # ---- static loads ----
            wq_t = []
            for k in range(16):
                t = singles.tile([128, 256], F32R, tag=f"wq{k}")
                nc.sync.dma_start(out=t, in_=wqt[k * 128:(k + 1) * 128, :])
                wq_t.append(t)
            wkv_t = []
            for k in range(16):
                t = singles.tile([128, 128], F32R, tag=f"wkv{k}")
                nc.sync.dma_start(out=t, in_=wkvt[k * 128:(k + 1) * 128, :])
                wkv_t.append(t)
            wo_t = []
            for i in range(2):
                t = singles.tile([128, 2048], F32R, tag=f"wo{i}")
                nc.sync.dma_start(out=t, in_=wot[i * 128:(i + 1) * 128, :])
                wo_t.append(t)
            cos_sb = singles.tile([128, S], F32, tag="cos")
            nc.sync.dma_start(out=cos_sb, in_=cos2)
            sin_sb = singles.tile([128, S], F32, tag="sin")
            nc.sync.dma_start(out=sin_sb, in_=sin2s)
            ones_sb = singles.tile([1, 64], F32R, tag="ones1")
            nc.sync.dma_start(out=ones_sb, in_=ones1)
            onescol_sb = singles.tile([128, 1], F32R, tag="onescol")
            nc.sync.dma_start(out=onescol_sb, in_=onescol)
            ident_sb = singles.tile([64, 64], F32R, tag="ident")
            nc.sync.dma_start(out=ident_sb, in_=ident)

            qTr0 = persist.tile([128, S], F32R, tag="qTr0")
            qTr1 = persist.tile([128, S], F32R, tag="qTr1")
            kTr = persist.tile([128, S], F32R, tag="kTr")  # rows 64:128 duplicate rows 0:64
            otp0 = persist.tile([128, S], F32R, tag="otp0")
            otp1 = persist.tile([128, S], F32R, tag="otp1")
            v1 = [singles.tile([128, 65], F32R, tag=f"v1_{kt}", name=f"v1_{kt}") for kt in range(16)]

            # ---- phase 1: projections + RoPE + v transposes ----
            for sc in range(8):
                scs = slice(sc * 256, (sc + 1) * 256)
                xts = []
                for k in range(16):
                    t = xtp.tile([128, 256], F32R, tag=f"x{k}")
                    nc.sync.dma_start(out=t, in_=xT[k * 128:(k + 1) * 128, scs])
                    xts.append(t)
                for ot in range(3):
                    acc = pp.tile([128, 256], F32, tag="acc")
                    for k in range(16):
                        lhsT = wq_t[k][:, ot * 128:(ot + 1) * 128] if ot < 2 else wkv_t[k]
                        nc.tensor.matmul(acc, lhsT, xts[k], start=(k == 0), stop=(k == 15))
                    t1 = rope.tile([128, 256], F32, tag="t1")
                    t2 = rope.tile([128, 256], F32, tag="t2")
                    if ot < 2:
                        nc.vector.tensor_mul(t1, acc, cos_sb[:, scs])
                        nc.vector.tensor_mul(t2[0:32], acc[32:64], sin_sb[0:32, scs])
                        nc.vector.tensor_mul(t2[32:64], acc[0:32], sin_sb[32:64, scs])
                        nc.vector.tensor_mul(t2[64:96], acc[96:128], sin_sb[64:96, scs])
                        nc.vector.tensor_mul(t2[96:128], acc[64:96], sin_sb[96:128, scs])
                        dst = qTr0 if ot == 0 else qTr1
                        nc.vector.tensor_add(dst[:, scs], t1, t2)
                    else:
                        nc.vector.tensor_mul(t1[0:64], acc[0:64], cos_sb[0:64, scs])
                        nc.vector.tensor_mul(t2[0:32], acc[32:64], sin_sb[0:32, scs])
                        nc.vector.tensor_mul(t2[32:64], acc[0:32], sin_sb[32:64, scs])
                        nc.vector.tensor_add(kTr[0:64, scs], t1[0:64], t2[0:64])
                        nc.vector.tensor_copy(kTr[64:128, scs], kTr[0:64, scs])
                        vtc = vtcp.tile([64, 256], F32R, tag="vtc")
                        nc.vector.tensor_copy(vtc, acc[64:128])
                        for b in range(2):
                            kt = sc * 2 + b
                            tp = pp.tile([128, 64], F32R, tag="acc")
                            nc.tensor.transpose(tp, vtc[:, b * 128:(b + 1) * 128], ident_sb)
                            nc.scalar.copy(v1[kt][:, 0:64], tp)
                            nc.scalar.copy(v1[kt][:, 64:65], onescol_sb)

            # ---- phase 2: attention + Wo per 512-wide q chunk ----
            for qc in range(4):
                qs = slice(qc * 512, (qc + 1) * 512)
                for hp in range(2):  # head pairs: scores hit row groups 0/64 -> concurrent MMs
                    qsrc = qTr0 if hp == 0 else qTr1
                    dst = otp0 if hp == 0 else otp1
                    pvA = pspv.tile([65, 512], F32, tag="pv", name=f"pvA_{qc}_{hp}")
                    pvB = pspv.tile([65, 512], F32, tag="pv", name=f"pvB_{qc}_{hp}")
                    for kt in range(16):
                        ktc = slice(kt * 128, (kt + 1) * 128)
                        spsA = pss.tile([128, 512], F32, tag="s", name=f"sA_{qc}_{hp}_{kt}")
                        nc.tensor.matmul(spsA, kTr[0:64, ktc], qsrc[0:64, qs],
                                         start=True, stop=True)
                        spsB = pss.tile([128, 512], F32, tag="s", name=f"sB_{qc}_{hp}_{kt}")
                        nc.tensor.matmul(spsB, kTr[64:128, ktc], qsrc[64:128, qs],
                                         start=True, stop=True)
                        ptA = ptp.tile([128, 512], F32R, tag="pt", name=f"ptA_{qc}_{hp}_{kt}")
                        nc.scalar.activation(ptA, spsA, EXP, scale=0.125)
                        nc.tensor.matmul(pvA, v1[kt], ptA, start=(kt == 0), stop=(kt == 15))
                        ptB = ptp.tile([128, 512], F32R, tag="pt", name=f"ptB_{qc}_{hp}_{kt}")
                        nc.scalar.activation(ptB, spsB, EXP, scale=0.125)
                        nc.tensor.matmul(pvB, v1[kt], ptB, start=(kt == 0), stop=(kt == 15))
                    for sub, pv in ((0, pvA), (1, pvB)):
                        hrow = sub * 64
                        rc = rcp.tile([1, 512], F32R, tag="rc", name=f"rc_{qc}_{hp}_{sub}")
                        nc.vector.reciprocal(rc, pv[64:65, :])
                        bps = psb.tile([64, 512], F32, tag="b", name=f"b_{qc}_{hp}_{sub}")
                        nc.tensor.matmul(bps, ones_sb, rc, start=True, stop=True)
                        bsb = othp.tile([64, 512], F32, tag="bsb", name=f"bsb_{qc}_{hp}_{sub}")
                        nc.vector.tensor_copy(bsb, bps)
                        oth = othp.tile([64, 512], F32R, tag="oth", name=f"oth_{qc}_{hp}_{sub}")
                        nc.vector.tensor_mul(oth, pv[0:64, :], bsb)
                        nc.scalar.dma_start(out=dst[hrow:hrow + 64, qs], in_=oth)
                for st in range(4):
                    sabs = qc * 4 + st
                    ss = slice(sabs * 128, (sabs + 1) * 128)
                    for mc in range(4):
                        ms = slice(mc * 512, (mc + 1) * 512)
                        yps = pp.tile([128, 512], F32, tag="acc")
                        nc.tensor.matmul(yps, otp0[:, ss], wo_t[0][:, ms], start=True, stop=False)
                        nc.tensor.matmul(yps, otp1[:, ss], wo_t[1][:, ms], start=False, stop=True)
                        ysb = ysbp.tile([128, 512], F32, tag="y")
                        nc.vector.tensor_copy(ysb, yps)
                        nc.gpsimd.dma_start(out=y[ss, ms], in_=ysb)

    nc.compile()
    return nc


def _host_prep(x, Wq, Wk, Wv, Wo):
    """Build per-core input maps (host-side numpy, untimed)."""
    x2 = np.ascontiguousarray(x.reshape(S, D), dtype=np.float32)
    xT = np.ascontiguousarray(x2.T)

    inv = 1.0 / (ROPE_BASE ** (np.arange(0, HD, 2, dtype=np.float32) / HD))
    t = np.arange(S, dtype=np.float32)
    ang = np.einsum("i,j->ij", t, inv)              # [S, 32]
    emb = np.concatenate([ang, ang], axis=-1)       # [S, 64]
    cosT = np.ascontiguousarray(np.cos(emb).T.astype(np.float32))   # [64, S]
    sinT = np.ascontiguousarray(np.sin(emb).T.astype(np.float32))
    sinTs = sinT.copy()
    sinTs[0:32] *= -1.0
    cos2 = np.ascontiguousarray(np.concatenate([cosT, cosT], axis=0))
    sin2s = np.ascontiguousarray(np.concatenate([sinTs, sinTs], axis=0))

    ones1 = np.ones((1, 64), dtype=np.float32)
    onescol = np.ones((128, 1), dtype=np.float32)
    ident = np.eye(64, dtype=np.float32)

    in_maps = []
    for c in range(N_CORES):
        osl = slice(c * 256, (c + 1) * 256)
        ksl = slice(c * 64, (c + 1) * 64)
        wqt = np.ascontiguousarray(Wq[osl, :].T.astype(np.float32))          # [D, 256]
        wkvt = np.ascontiguousarray(
            np.concatenate([Wk[ksl, :], Wv[ksl, :]], axis=0).T.astype(np.float32))  # [D, 128]
        wot = np.ascontiguousarray(Wo[:, osl].T.astype(np.float32))          # [256, D]
        in_maps.append({
            "xT": xT, "wqt": wqt, "wkvt": wkvt, "wot": wot,
            "cos2": cos2, "sin2s": sin2s,
            "ones1": ones1, "onescol": onescol, "ident": ident,
        })
    return in_maps


def kernel(x, Wq, Wk, Wv, Wo, _trace=False):
    from concourse.bass_utils import run_bass_kernel_spmd

    x = np.asarray(x, dtype=np.float32)
    Wq = np.asarray(Wq, dtype=np.float32)
    Wk = np.asarray(Wk, dtype=np.float32)
    Wv = np.asarray(Wv, dtype=np.float32)
    Wo = np.asarray(Wo, dtype=np.float32)

    if "nc" not in _cached:
        _cached["nc"] = _build_program()
    nc = _cached["nc"]

    in_maps = _host_prep(x, Wq, Wk, Wv, Wo)
    res = run_bass_kernel_spmd(nc, in_maps, core_ids=list(range(N_CORES)),
                               trace=_trace)
    out = np.zeros((S, D), dtype=np.float64)
    for r in res.results:
        out += r["y"].astype(np.float64)
    _cached["last_results"] = res
    return out.astype(np.float32).reshape(1, S, D)
